# revision 25
# baseline (speedup 1.0000x reference)
"""Trainium2 Bass kernel: NeuralNearestNeighbors continuous-KNN weight volumes.

Reference computation (per row of D.reshape(b*m, o), K=8 rounds):
    logits = D / exp(log_temp)
    for k in range(K):
        w_k = log_softmax(logits);  out_k = exp(w_k)
        logits = logits + log1mexp(w_k)          # log(1 - p_k)
    W = stack(out_k, axis=-1)                     # (b, m, o, K)

Scale-invariant recurrence used on device: keep a state S that is an
arbitrary per-row scalar multiple of the round's softmax weights F_k,
with its true row-sum `a = sum(S)` tracked by each op's accumulator.
Then F_k = S*(1/a) always (since sum(F_k) == 1), and

    S' = (S - a)*S          a' = sum(S')

reproduces the reference exactly (S' = a^2*(F-F^2), and the scale a^2
cancels in the next normalization).  One native DVE scalar_tensor_tensor
per round with the accumulator fed back as the per-partition scalar -
no reciprocal on the chain.  The state squares each round, so it is
rescaled to F (one tensor_scalar by gam=1/a) at rounds {1,4} to stay in
fp16 range; after a rescale |S| <= ~50 for the next two rounds.

Engine/dtype assignment (measured on HW, per [128, 512] op):
  - chain stt fp16 contiguous on DVE: ~729 ns (2x 2-byte DVE mode)
  - strided (stride-8) f32 output writes: DVE tensor_scalar ~683 ns,
    ACT activation-copy ~1270 ns; split ACT:DVE = 23:9 per wave of 4
    tiles x 8 rounds.  fp16 strided writes are 2-4x slower than f32 on
    both engines, and GpSimd strided writes are ~7.8 us - never used.
  - exp on ACT (fp16 out, f32 accum, bias -6 bounds pre-rescale growth)
  - batched [P,4] reciprocals per wave round on DVE (off-chain, feeds
    only the strided output writes).
Waves of 4 tiles are software-pipelined (next wave's exps are emitted
mid-wave) so ACT/DVE stay >90% busy.  Output tile stays f32 (fp16
strided writes would cost more engine time than the DMA saves).

Sharding: purely rowwise data-parallel over b*m = 16384 rows; 2048 rows
per core across 8 cores; log_temp replicated.  HW exec ~152 us vs 303 us
baseline; DVE ~138 us / ACT ~145 us busy, DMA ~115 us.
"""

import os

import numpy as np

B, M, O = 16, 1024, 512
K = 8
N_CORES = 8
ROWS = B * M                     # 16384
RPC = ROWS // N_CORES            # 2048 rows per core
P = 128
TILES = RPC // P                 # 16 row-tiles per core
WV = 4                           # tiles per wave
WAVES = TILES // WV

VARIANT = os.environ.get("KVAR", "hybr2")

# variant -> (state_dtype, out_dtype, chain, pass1 engine counts (A, P, D))
_CFG = {
    "f32a": ("f32", "f32", "amr", (20, 6, 6)),
    "f32n": ("f32", "f32", "natr", (20, 6, 6)),
    "mx1": ("f32", "f32", "natr", (16, 0, 16)),
    "h16a": ("f16", "f16", "amr", (15, 13, 4)),
    "h16n": ("f16", "f16", "nat", (26, 0, 6)),
    "hyb": ("f16", "f32", "nat", (15, 0, 17)),
    "hybr": ("f16", "f32", "natr", (11, 0, 21)),
    "hybr2": ("f16", "f32", "natr", (23, 0, 9)),
    "hybr3": ("f16", "f32", "natr", (22, 0, 10)),
    "final": ("f16", "f32", "natr", (21, 0, 11)),
    "hybr4": ("f16", "f32", "natr", (24, 0, 8)),
    "sq": ("f16", "f32", "natr", (17, 0, 15)),
    "sq1": ("f16", "f32", "natr", (20, 0, 12)),
    "pe": ("f16", "f16", "pe", (27, 0, 5)),
}

_cached = {}


def _make_pattern(n_act, n_pool, n_dve, total):
    """Largest-remainder round-robin spread of engine codes over slots."""
    pools = [("A", n_act), ("P", n_pool), ("D", n_dve)]
    credit = {c: 0.0 for c, _ in pools}
    out = []
    for _ in range(total):
        for c, n in pools:
            credit[c] += n / total
        pick = max(credit, key=lambda c: credit[c])
        credit[pick] -= 1.0
        out.append(pick)
    return out


def _build(variant):
    from contextlib import ExitStack

    import concourse.bacc as bacc
    import concourse.tile as tile
    from concourse import mybir

    f32 = mybir.dt.float32
    f16 = mybir.dt.float16
    Alu = mybir.AluOpType
    Act = mybir.ActivationFunctionType

    sdt_s, odt_s, chain, counts = _CFG[variant]
    sdt = f16 if sdt_s == "f16" else f32
    odt = f16 if odt_s == "f16" else f32
    pat = _make_pattern(*counts, total=K * WV)

    nc = bacc.Bacc(
        "TRN2",
        target_bir_lowering=False,
        debug=False,
        enable_asserts=False,
        num_devices=N_CORES,
    )
    d = nc.dram_tensor("d", [RPC, O], f32, kind="ExternalInput").ap()
    lt = nc.dram_tensor("log_temp", [1, 1], f32, kind="ExternalInput").ap()
    w = nc.dram_tensor("w", [RPC, O * K], odt, kind="ExternalOutput").ap()

    with tile.TileContext(nc) as tc, ExitStack() as ctx:
        singles = ctx.enter_context(tc.tile_pool(name="singles", bufs=1))
        dpool = ctx.enter_context(tc.tile_pool(name="dslab", bufs=1))
        gpool = ctx.enter_context(
            tc.tile_pool(name="state", bufs=20 if chain in ("nat", "natr") else 16)
        )
        outp = ctx.enter_context(
            tc.tile_pool(name="out", bufs=10 if odt == f16 else 7)
        )
        small = ctx.enter_context(tc.tile_pool(name="small", bufs=64))
        if chain == "pe":
            fpool = ctx.enter_context(tc.tile_pool(name="fbuf", bufs=72))
            pspool = ctx.enter_context(tc.psum_pool(name="ps", bufs=2))

        # log_temp -> 1/T = exp(-log_temp), replicated to all 128 partitions.
        lt_sb = singles.tile([P, 1], f32)
        nc.sync.dma_start(out=lt_sb[:, :], in_=lt.to_broadcast((P, 1)))
        invt = singles.tile([P, 1], f32)
        nc.scalar.activation(invt[:, :], lt_sb[:, :], Act.Exp, scale=-1.0)
        bias6 = singles.tile([P, 1], f32)
        nc.vector.memset(bias6[:, :], -6.0)
        biash = singles.tile([P, 1], f32)
        nc.vector.memset(biash[:, :], -0.5)
        if chain == "pe":
            # fp16 identity for PE copy-through: I[p, j] = (p == j)
            ident = singles.tile([P, P], f16)
            nc.gpsimd.memset(ident[:, :], 1.0)
            nc.gpsimd.affine_select(
                out=ident[:, :],
                in_=ident[:, :],
                compare_op=Alu.is_equal,
                fill=0.0,
                base=0,
                pattern=[[-1, P]],
                channel_multiplier=1,
            )

        din = d.rearrange("(t p) o -> p t o", p=P)
        dslab = dpool.tile([P, TILES, O], f32)
        for g in range(WAVES):
            # SWDGE path keeps the HWDGE rings free for output writes.
            nc.gpsimd.dma_start(
                out=dslab[:, g * WV : (g + 1) * WV, :],
                in_=din[:, g * WV : (g + 1) * WV, :],
            )

        wave_state = {}

        def emit_exps(g):
            """exp round for wave g: S_0 = exp(D/T), acc = row sums."""
            acc = small.tile([P, WV], f32)
            S = []
            for i in range(WV):
                t = g * WV + i
                s0 = gpool.tile([P, O], sdt, name="st")
                nc.scalar.activation(
                    s0[:, :],
                    dslab[:, t, :],
                    Act.Exp,
                    scale=invt[:, :],
                    bias=bias6[:, :],
                    accum_out=acc[:, i : i + 1],
                )
                S.append(s0)
            wave_state[g] = (S, acc)

        def emit_rounds_pe(g):
            S, acc = wave_state.pop(g)
            fcs = [[None] * K for _ in range(WV)]
            for r in range(K):
                gam = small.tile([P, WV], f32)
                nc.vector.reciprocal(gam[:, :], acc[:, :])
                for i in range(WV):
                    gi = gam[:, i : i + 1]
                    fc = fpool.tile([P, O], f16, name="fc")
                    e = pat[r * WV + i]
                    if e == "D":
                        nc.vector.tensor_scalar(
                            fc[:, :], S[i][:, :], gi, None, Alu.mult
                        )
                    else:
                        nc.scalar.mul(fc[:, :], S[i][:, :], gi)
                    fcs[i][r] = fc
                if r == 2 and g + 1 < WAVES:
                    emit_exps(g + 1)
                if r == K - 1:
                    break
                accn = small.tile([P, WV], f32)
                for i in range(WV):
                    # chain: S' = (S - a)*S with accumulator feedback; at
                    # rounds 1 and 4 restart from the normalized F (already
                    # materialized for the PE) to keep fp16 bounded.
                    if r in (1, 4):
                        src_t, scal = fcs[i][r], 1.0
                    else:
                        src_t, scal = S[i], acc[:, i : i + 1]
                    sn = gpool.tile([P, O], sdt, name="st")
                    nc.vector.scalar_tensor_tensor(
                        out=sn[:, :],
                        in0=src_t[:, :],
                        scalar=scal,
                        in1=src_t[:, :],
                        op0=Alu.subtract,
                        op1=Alu.mult,
                        accum_out=accn[:, i : i + 1],
                    )
                    S[i] = sn
                acc = accn
            for i in range(WV):
                t = g * WV + i
                ot = outp.tile([P, O, K], odt, name="ot")
                for h in range(2):
                    ph = pspool.tile([P, O // 2, K], f32, name="ph")
                    for k in range(K):
                        nc.tensor.matmul(
                            out=ph[:, :, k],
                            lhsT=ident[:, :],
                            rhs=fcs[i][k][:, h * (O // 2) : (h + 1) * (O // 2)],
                            start=True,
                            stop=True,
                        )
                    # PSUM -> SBUF fp16 downcast on the idle GpSimd engine
                    nc.gpsimd.tensor_copy(
                        ot[:, h * (O // 2) : (h + 1) * (O // 2), :],
                        ph[:, :, :],
                    )
                nc.sync.dma_start(
                    out=w[t * P : (t + 1) * P, :], in_=ot[:, :, :]
                )

        def emit_rounds_sq(g):
            """ACT-square chain wave: state v = (F-0.5)^2 (f32), G' = 0.25-v.
            Round update is one ACT Square op with [P,1] scale/bias APs; the
            whole recurrence runs on ACT, freeing DVE for strided writes."""
            S, acc = wave_state.pop(g)
            patq = _make_pattern(11, 0, 21, K * WV)
            outs = [outp.tile([P, O, K], odt, name="ot") for _ in range(WV)]
            V = [None] * WV
            for r in range(K):
                if r == 0:
                    gam = small.tile([P, WV], f32)
                    nc.vector.reciprocal(gam[:, :], acc[:, :])
                else:
                    # acc holds sum(v); a' = 128 - sum(v); use gneg = -1/a'
                    am = small.tile([P, WV], f32)
                    nc.vector.tensor_scalar(
                        am[:, :], acc[:, :], 1.0, -128.0, Alu.mult, Alu.add
                    )
                    gam = small.tile([P, WV], f32)
                    nc.vector.reciprocal(gam[:, :], am[:, :])  # = -1/a'
                    # pass1 offset t1 = 0.25*gam' = -0.25*gneg; square bias
                    # bg = t1 - 0.5
                    t1 = small.tile([P, WV], f32)
                    nc.vector.tensor_scalar(
                        t1[:, :], gam[:, :], -0.25, None, Alu.mult
                    )
                    bg = small.tile([P, WV], f32)
                    nc.vector.tensor_scalar(
                        bg[:, :], t1[:, :], 1.0, -0.5, Alu.mult, Alu.add
                    )
                for i in range(WV):
                    f = outs[i][:, :, r]
                    gi = gam[:, i : i + 1]
                    e = patq[r * WV + i]
                    if r == 0:
                        # F_0 = S*gam
                        if e == "D":
                            nc.vector.tensor_scalar(
                                f, S[i][:, :], gi, None, Alu.mult
                            )
                        else:
                            nc.scalar.mul(f, S[i][:, :], gi)
                    else:
                        # F_r = (0.25 - v)*gam' = v*gneg + t1; Copy cannot
                        # take an AP bias, so these always run on DVE
                        nc.vector.tensor_scalar(
                            f, V[i][:, :], gi, t1[:, i : i + 1],
                            Alu.mult, Alu.add,
                        )
                if r == 2 and g + 1 < WAVES:
                    emit_exps(g + 1)
                if r == K - 1:
                    break
                accn = small.tile([P, WV], f32)
                for i in range(WV):
                    # v' = (F - 0.5)^2, accum sum(v'); F expressed via the
                    # same scale/bias as pass1 (r=0: S*gam - 0.5)
                    vn = gpool.tile([P, O], f32, name="vt", bufs=10)
                    if r == 0:
                        nc.scalar.activation(
                            vn[:, :], S[i][:, :], Act.Square,
                            bias=biash[:, :], scale=gam[:, i : i + 1],
                            accum_out=accn[:, i : i + 1],
                        )
                    else:
                        nc.scalar.activation(
                            vn[:, :], V[i][:, :], Act.Square,
                            bias=bg[:, i : i + 1], scale=gam[:, i : i + 1],
                            accum_out=accn[:, i : i + 1],
                        )
                    V[i] = vn
                acc = accn
            for i in range(WV):
                t = g * WV + i
                nc.sync.dma_start(
                    out=w[t * P : (t + 1) * P, :], in_=outs[i][:, :, :]
                )

        def emit_rounds(g):
            if chain == "pe":
                return emit_rounds_pe(g)
            if (variant == "sq" and g % 2 == 1) or (variant == "sq1" and g == 1):
                return emit_rounds_sq(g)
            S, acc = wave_state.pop(g)
            outs = [outp.tile([P, O, K], odt, name="ot") for _ in range(WV)]
            for r in range(K):
                gam = small.tile([P, WV], f32)
                nc.vector.reciprocal(gam[:, :], acc[:, :])
                for i in range(WV):
                    f = outs[i][:, :, r]
                    gi = gam[:, i : i + 1]
                    e = pat[r * WV + i]
                    if e == "A":
                        nc.scalar.mul(f, S[i][:, :], gi)
                    elif e == "D":
                        nc.vector.tensor_scalar(f, S[i][:, :], gi, None, Alu.mult)
                    else:
                        nc.gpsimd.tensor_scalar(f, S[i][:, :], gi, None, Alu.mult)
                if r == 2 and g + 1 < WAVES:
                    # software pipeline: next wave's exps land on ACT now so
                    # its first reciprocal is ready at this wave's end.
                    emit_exps(g + 1)
                if r == K - 1:
                    break
                accn = small.tile([P, WV], f32)
                for i in range(WV):
                    gi = gam[:, i : i + 1]
                    if chain == "amr":
                        sn = gpool.tile([P, O], sdt, name="st")
                        nc.vector.affine_mul_reduce(
                            out=sn[:, :],
                            accum_out=accn[:, i : i + 1],
                            in0=S[i][:, :],
                            in1=S[i][:, :],
                            scale=gi,
                            bias=-1.0,
                        )
                    elif chain == "natr":
                        # native chain: S' = (S - a)*S is exact for any state
                        # scale; rescale keeps the state's squared-growth
                        # bounded (f32: once; fp16: twice, max |S| ~50).
                        rescale_rounds = (1, 4) if sdt == f16 else (2,)
                        if r in rescale_rounds:
                            fc = gpool.tile([P, O], sdt, name="st")
                            reng = nc.vector
                            reng.tensor_scalar(
                                fc[:, :], S[i][:, :], gi, None, Alu.mult
                            )
                            src_t, scal = fc, 1.0
                        else:
                            src_t, scal = S[i], acc[:, i : i + 1]
                        sn = gpool.tile([P, O], sdt, name="st")
                        ceng = nc.vector
                        if variant == "mx1" and i % 4 != 0:
                            ceng = nc.gpsimd
                        ceng.scalar_tensor_tensor(
                            out=sn[:, :],
                            in0=src_t[:, :],
                            scalar=scal,
                            in1=src_t[:, :],
                            op0=Alu.subtract,
                            op1=Alu.mult,
                            accum_out=accn[:, i : i + 1],
                        )
                    else:
                        fc = gpool.tile([P, O], sdt, name="st")
                        nc.vector.tensor_scalar(
                            fc[:, :], S[i][:, :], gi, None, Alu.mult
                        )
                        sn = gpool.tile([P, O], sdt, name="st")
                        nc.vector.scalar_tensor_tensor(
                            out=sn[:, :],
                            in0=fc[:, :],
                            scalar=1.0,
                            in1=fc[:, :],
                            op0=Alu.subtract,
                            op1=Alu.mult,
                            accum_out=accn[:, i : i + 1],
                        )
                    S[i] = sn
                acc = accn
            if variant == "probe":
                # probe: stride-2 fp16 write cost (for pair-pack evaluation)
                pb = gpool.tile([P, O, 2], f16, name="pb", bufs=2)
                nc.vector.tensor_scalar(
                    pb[:, :, 0], S[0][:, :], gam[:, 0:1], None, Alu.mult
                )
                nc.vector.tensor_scalar(
                    pb[:, :, 1], S[1][:, :], gam[:, 1:2], None, Alu.mult
                )
                nc.scalar.mul(pb[:, :, 0], S[2][:, :], gam[:, 2:3])
            for i in range(WV):
                t = g * WV + i
                nc.sync.dma_start(
                    out=w[t * P : (t + 1) * P, :], in_=outs[i][:, :, :]
                )

        emit_exps(0)
        for g in range(WAVES):
            emit_rounds(g)

    nc.compile()
    return nc


def _get_nc(variant=None):
    variant = variant or VARIANT
    if variant not in _cached:
        _cached[variant] = _build(variant)
    return _cached[variant]


def _make_in_maps(D, log_temp):
    Dr = np.ascontiguousarray(np.asarray(D, dtype=np.float32).reshape(ROWS, O))
    lt = np.asarray(log_temp, dtype=np.float32).reshape(1, 1)
    return [
        {"d": Dr[c * RPC : (c + 1) * RPC], "log_temp": lt}
        for c in range(N_CORES)
    ]


def _gather(results):
    parts = [
        np.asarray(results[c]["w"], dtype=np.float32).reshape(RPC, O, K)
        for c in range(N_CORES)
    ]
    return np.concatenate(parts, axis=0).reshape(B, M, O, K)


def run_spmd(D, log_temp, trace=False, variant=None, **kwargs):
    """Run on all 8 cores; returns (W, BassKernelResults)."""
    from concourse.bass_utils import run_bass_kernel_spmd

    nc = _get_nc(variant)
    res = run_bass_kernel_spmd(
        nc, _make_in_maps(D, log_temp), list(range(N_CORES)), trace=trace, **kwargs
    )
    return _gather(res.results), res


def kernel(D, log_temp):
    W, _ = run_spmd(D, log_temp)
    return W


# revision 26
# speedup vs baseline: 1.2866x; 1.2866x over previous
"""Trainium2 Bass kernel: NeuralNearestNeighbors continuous-KNN weight volumes.

Reference computation (per row of D.reshape(b*m, o), K=8 rounds):
    logits = D / exp(log_temp)
    for k in range(K):
        w_k = log_softmax(logits);  out_k = exp(w_k)
        logits = logits + log1mexp(w_k)          # log(1 - p_k)
    W = stack(out_k, axis=-1)                     # (b, m, o, K)

Scale-invariant recurrence used on device: keep a state S that is an
arbitrary per-row scalar multiple of the round's softmax weights F_k,
with its true row-sum `a = sum(S)` tracked by each op's accumulator.
Then F_k = S*(1/a) always (since sum(F_k) == 1), and

    S' = (S - a)*S          a' = sum(S')

reproduces the reference exactly (S' = a^2*(F-F^2), and the scale a^2
cancels in the next normalization).  One native DVE scalar_tensor_tensor
per round with the accumulator fed back as the per-partition scalar -
no reciprocal on the chain.  The state squares each round, so it is
rescaled to F (one tensor_scalar by gam=1/a) at rounds {1,4} to stay in
fp16 range; after a rescale |S| <= ~50 for the next two rounds.

Engine/dtype assignment (measured on HW, per [128, 512] op):
  - chain stt fp16 contiguous on DVE: ~729 ns (2x 2-byte DVE mode)
  - strided (stride-8) f32 output writes: DVE tensor_scalar ~683 ns,
    ACT activation-copy ~1270 ns; split ACT:DVE = 23:9 per wave of 4
    tiles x 8 rounds.  fp16 strided writes are 2-4x slower than f32 on
    both engines, and GpSimd strided writes are ~7.8 us - never used.
  - exp on ACT (fp16 out, f32 accum, bias -6 bounds pre-rescale growth)
  - batched [P,4] reciprocals per wave round on DVE (off-chain, feeds
    only the strided output writes).
Waves of 4 tiles are software-pipelined (next wave's exps are emitted
mid-wave) so ACT/DVE stay >90% busy.  Output tile stays f32 (fp16
strided writes would cost more engine time than the DMA saves).

Sharding: purely rowwise data-parallel over b*m = 16384 rows; 2048 rows
per core across 8 cores; log_temp replicated.  HW exec ~152 us vs 303 us
baseline; DVE ~138 us / ACT ~145 us busy, DMA ~115 us.
"""

import os

import numpy as np

B, M, O = 16, 1024, 512
K = 8
N_CORES = 8
ROWS = B * M                     # 16384
RPC = ROWS // N_CORES            # 2048 rows per core
P = 128
TILES = RPC // P                 # 16 row-tiles per core
WV = 4                           # tiles per wave
WAVES = TILES // WV

VARIANT = os.environ.get("KVAR", "hybr2")

# variant -> (state_dtype, out_dtype, chain, pass1 engine counts (A, P, D))
_CFG = {
    "f32a": ("f32", "f32", "amr", (20, 6, 6)),
    "f32n": ("f32", "f32", "natr", (20, 6, 6)),
    "mx1": ("f32", "f32", "natr", (16, 0, 16)),
    "h16a": ("f16", "f16", "amr", (15, 13, 4)),
    "h16n": ("f16", "f16", "nat", (26, 0, 6)),
    "hyb": ("f16", "f32", "nat", (15, 0, 17)),
    "hybr": ("f16", "f32", "natr", (11, 0, 21)),
    "hybr2": ("f16", "f32", "natr", (23, 0, 9)),
    "hybr3": ("f16", "f32", "natr", (22, 0, 10)),
    "final": ("f16", "f32", "natr", (21, 0, 11)),
    "hybr4": ("f16", "f32", "natr", (24, 0, 8)),
    "hybr5": ("f16", "f32", "natr", (23, 0, 9)),
    "sq": ("f16", "f32", "natr", (17, 0, 15)),
    "sq1": ("f16", "f32", "natr", (20, 0, 12)),
    "pe": ("f16", "f16", "pe", (27, 0, 5)),
}

_cached = {}


def _make_pattern(n_act, n_pool, n_dve, total):
    """Largest-remainder round-robin spread of engine codes over slots."""
    pools = [("A", n_act), ("P", n_pool), ("D", n_dve)]
    credit = {c: 0.0 for c, _ in pools}
    out = []
    for _ in range(total):
        for c, n in pools:
            credit[c] += n / total
        pick = max(credit, key=lambda c: credit[c])
        credit[pick] -= 1.0
        out.append(pick)
    return out


def _build(variant):
    from contextlib import ExitStack

    import concourse.bacc as bacc
    import concourse.tile as tile
    from concourse import mybir

    f32 = mybir.dt.float32
    f16 = mybir.dt.float16
    Alu = mybir.AluOpType
    Act = mybir.ActivationFunctionType

    sdt_s, odt_s, chain, counts = _CFG[variant]
    sdt = f16 if sdt_s == "f16" else f32
    odt = f16 if odt_s == "f16" else f32
    pat = _make_pattern(*counts, total=K * WV)

    nc = bacc.Bacc(
        "TRN2",
        target_bir_lowering=False,
        debug=False,
        enable_asserts=False,
        num_devices=N_CORES,
    )
    d = nc.dram_tensor("d", [RPC, O], f32, kind="ExternalInput").ap()
    lt = nc.dram_tensor("log_temp", [1, 1], f32, kind="ExternalInput").ap()
    w = nc.dram_tensor("w", [RPC, O * K], odt, kind="ExternalOutput").ap()

    with tile.TileContext(nc) as tc, ExitStack() as ctx:
        singles = ctx.enter_context(tc.tile_pool(name="singles", bufs=1))
        dpool = ctx.enter_context(tc.tile_pool(name="dslab", bufs=1))
        gpool = ctx.enter_context(
            tc.tile_pool(name="state", bufs=20 if variant in ("sq", "sq1") else (28 if chain in ("nat", "natr") else 16))
        )
        outp = ctx.enter_context(
            tc.tile_pool(name="out", bufs=10 if odt == f16 else 7)
        )
        small = ctx.enter_context(tc.tile_pool(name="small", bufs=64))
        if chain == "pe":
            fpool = ctx.enter_context(tc.tile_pool(name="fbuf", bufs=72))
            pspool = ctx.enter_context(tc.psum_pool(name="ps", bufs=2))

        # log_temp -> 1/T = exp(-log_temp), replicated to all 128 partitions.
        lt_sb = singles.tile([P, 1], f32)
        nc.sync.dma_start(out=lt_sb[:, :], in_=lt.to_broadcast((P, 1)))
        invt = singles.tile([P, 1], f32)
        nc.scalar.activation(invt[:, :], lt_sb[:, :], Act.Exp, scale=-1.0)
        bias6 = singles.tile([P, 1], f32)
        nc.vector.memset(bias6[:, :], -6.0)
        biash = singles.tile([P, 1], f32)
        nc.vector.memset(biash[:, :], -0.5)
        if chain == "pe":
            # fp16 identity for PE copy-through: I[p, j] = (p == j)
            ident = singles.tile([P, P], f16)
            nc.gpsimd.memset(ident[:, :], 1.0)
            nc.gpsimd.affine_select(
                out=ident[:, :],
                in_=ident[:, :],
                compare_op=Alu.is_equal,
                fill=0.0,
                base=0,
                pattern=[[-1, P]],
                channel_multiplier=1,
            )

        din = d.rearrange("(t p) o -> p t o", p=P)
        dslab = dpool.tile([P, TILES, O], f32)
        if variant == "hybr5":
            nc.gpsimd.dma_start(out=dslab[:, 0:1, :], in_=din[:, 0:1, :])
            nc.gpsimd.dma_start(out=dslab[:, 1:WV, :], in_=din[:, 1:WV, :])
        for g in range(1 if variant == "hybr5" else 0, WAVES):
            # SWDGE path keeps the HWDGE rings free for output writes.
            nc.gpsimd.dma_start(
                out=dslab[:, g * WV : (g + 1) * WV, :],
                in_=din[:, g * WV : (g + 1) * WV, :],
            )

        wave_state = {}

        def emit_exps(g):
            """exp round for wave g: S_0 = exp(D/T), acc = row sums."""
            acc = small.tile([P, WV], f32)
            S = []
            for i in range(WV):
                t = g * WV + i
                s0 = gpool.tile([P, O], sdt, name="st")
                nc.scalar.activation(
                    s0[:, :],
                    dslab[:, t, :],
                    Act.Exp,
                    scale=invt[:, :],
                    bias=bias6[:, :],
                    accum_out=acc[:, i : i + 1],
                )
                S.append(s0)
            wave_state[g] = (S, acc)

        def emit_rounds_pe(g):
            S, acc = wave_state.pop(g)
            fcs = [[None] * K for _ in range(WV)]
            for r in range(K):
                gam = small.tile([P, WV], f32)
                nc.vector.reciprocal(gam[:, :], acc[:, :])
                for i in range(WV):
                    gi = gam[:, i : i + 1]
                    fc = fpool.tile([P, O], f16, name="fc")
                    e = pat[r * WV + i]
                    if e == "D":
                        nc.vector.tensor_scalar(
                            fc[:, :], S[i][:, :], gi, None, Alu.mult
                        )
                    else:
                        nc.scalar.mul(fc[:, :], S[i][:, :], gi)
                    fcs[i][r] = fc
                if r == 2 and g + 1 < WAVES:
                    emit_exps(g + 1)
                if r == K - 1:
                    break
                accn = small.tile([P, WV], f32)
                for i in range(WV):
                    # chain: S' = (S - a)*S with accumulator feedback; at
                    # rounds 1 and 4 restart from the normalized F (already
                    # materialized for the PE) to keep fp16 bounded.
                    if r in (1, 4):
                        src_t, scal = fcs[i][r], 1.0
                    else:
                        src_t, scal = S[i], acc[:, i : i + 1]
                    sn = gpool.tile([P, O], sdt, name="st")
                    nc.vector.scalar_tensor_tensor(
                        out=sn[:, :],
                        in0=src_t[:, :],
                        scalar=scal,
                        in1=src_t[:, :],
                        op0=Alu.subtract,
                        op1=Alu.mult,
                        accum_out=accn[:, i : i + 1],
                    )
                    S[i] = sn
                acc = accn
            for i in range(WV):
                t = g * WV + i
                ot = outp.tile([P, O, K], odt, name="ot")
                for h in range(2):
                    ph = pspool.tile([P, O // 2, K], f32, name="ph")
                    for k in range(K):
                        nc.tensor.matmul(
                            out=ph[:, :, k],
                            lhsT=ident[:, :],
                            rhs=fcs[i][k][:, h * (O // 2) : (h + 1) * (O // 2)],
                            start=True,
                            stop=True,
                        )
                    # PSUM -> SBUF fp16 downcast on the idle GpSimd engine
                    nc.gpsimd.tensor_copy(
                        ot[:, h * (O // 2) : (h + 1) * (O // 2), :],
                        ph[:, :, :],
                    )
                nc.sync.dma_start(
                    out=w[t * P : (t + 1) * P, :], in_=ot[:, :, :]
                )

        def emit_rounds_sq(g):
            """ACT-square chain wave: state v = (F-0.5)^2 (f32), G' = 0.25-v.
            Round update is one ACT Square op with [P,1] scale/bias APs; the
            whole recurrence runs on ACT, freeing DVE for strided writes."""
            S, acc = wave_state.pop(g)
            patq = _make_pattern(11, 0, 21, K * WV)
            outs = [outp.tile([P, O, K], odt, name="ot") for _ in range(WV)]
            V = [None] * WV
            for r in range(K):
                if r == 0:
                    gam = small.tile([P, WV], f32)
                    nc.vector.reciprocal(gam[:, :], acc[:, :])
                else:
                    # acc holds sum(v); a' = 128 - sum(v); use gneg = -1/a'
                    am = small.tile([P, WV], f32)
                    nc.vector.tensor_scalar(
                        am[:, :], acc[:, :], 1.0, -128.0, Alu.mult, Alu.add
                    )
                    gam = small.tile([P, WV], f32)
                    nc.vector.reciprocal(gam[:, :], am[:, :])  # = -1/a'
                    # pass1 offset t1 = 0.25*gam' = -0.25*gneg; square bias
                    # bg = t1 - 0.5
                    t1 = small.tile([P, WV], f32)
                    nc.vector.tensor_scalar(
                        t1[:, :], gam[:, :], -0.25, None, Alu.mult
                    )
                    bg = small.tile([P, WV], f32)
                    nc.vector.tensor_scalar(
                        bg[:, :], t1[:, :], 1.0, -0.5, Alu.mult, Alu.add
                    )
                for i in range(WV):
                    f = outs[i][:, :, r]
                    gi = gam[:, i : i + 1]
                    e = patq[r * WV + i]
                    if r == 0:
                        # F_0 = S*gam
                        if e == "D":
                            nc.vector.tensor_scalar(
                                f, S[i][:, :], gi, None, Alu.mult
                            )
                        else:
                            nc.scalar.mul(f, S[i][:, :], gi)
                    else:
                        # F_r = (0.25 - v)*gam' = v*gneg + t1; Copy cannot
                        # take an AP bias, so these always run on DVE
                        nc.vector.tensor_scalar(
                            f, V[i][:, :], gi, t1[:, i : i + 1],
                            Alu.mult, Alu.add,
                        )
                if r == 2 and g + 1 < WAVES:
                    emit_exps(g + 1)
                if r == K - 1:
                    break
                accn = small.tile([P, WV], f32)
                for i in range(WV):
                    # v' = (F - 0.5)^2, accum sum(v'); F expressed via the
                    # same scale/bias as pass1 (r=0: S*gam - 0.5)
                    vn = gpool.tile([P, O], f32, name="vt", bufs=10)
                    if r == 0:
                        nc.scalar.activation(
                            vn[:, :], S[i][:, :], Act.Square,
                            bias=biash[:, :], scale=gam[:, i : i + 1],
                            accum_out=accn[:, i : i + 1],
                        )
                    else:
                        nc.scalar.activation(
                            vn[:, :], V[i][:, :], Act.Square,
                            bias=bg[:, i : i + 1], scale=gam[:, i : i + 1],
                            accum_out=accn[:, i : i + 1],
                        )
                    V[i] = vn
                acc = accn
            for i in range(WV):
                t = g * WV + i
                nc.sync.dma_start(
                    out=w[t * P : (t + 1) * P, :], in_=outs[i][:, :, :]
                )

        def emit_rounds(g):
            if chain == "pe":
                return emit_rounds_pe(g)
            if (variant == "sq" and g % 2 == 1) or (variant == "sq1" and g == 1):
                return emit_rounds_sq(g)
            S, acc = wave_state.pop(g)
            outs = [outp.tile([P, O, K], odt, name="ot") for _ in range(WV)]
            for r in range(K):
                gam = small.tile([P, WV], f32)
                nc.vector.reciprocal(gam[:, :], acc[:, :])
                for i in range(WV):
                    f = outs[i][:, :, r]
                    gi = gam[:, i : i + 1]
                    e = pat[r * WV + i]
                    if e == "A":
                        nc.scalar.mul(f, S[i][:, :], gi)
                    elif e == "D":
                        nc.vector.tensor_scalar(f, S[i][:, :], gi, None, Alu.mult)
                    else:
                        nc.gpsimd.tensor_scalar(f, S[i][:, :], gi, None, Alu.mult)
                if r == 2 and g + 1 < WAVES:
                    # software pipeline: next wave's exps land on ACT now so
                    # its first reciprocal is ready at this wave's end.
                    emit_exps(g + 1)
                if r == K - 1:
                    break
                accn = small.tile([P, WV], f32)
                for i in range(WV):
                    gi = gam[:, i : i + 1]
                    if chain == "amr":
                        sn = gpool.tile([P, O], sdt, name="st")
                        nc.vector.affine_mul_reduce(
                            out=sn[:, :],
                            accum_out=accn[:, i : i + 1],
                            in0=S[i][:, :],
                            in1=S[i][:, :],
                            scale=gi,
                            bias=-1.0,
                        )
                    elif chain == "natr":
                        # native chain: S' = (S - a)*S is exact for any state
                        # scale; rescale keeps the state's squared-growth
                        # bounded (f32: once; fp16: twice, max |S| ~50).
                        rescale_rounds = (1, 4) if sdt == f16 else (2,)
                        if r in rescale_rounds:
                            fc = gpool.tile([P, O], sdt, name="st")
                            reng = nc.vector
                            reng.tensor_scalar(
                                fc[:, :], S[i][:, :], gi, None, Alu.mult
                            )
                            src_t, scal = fc, 1.0
                        else:
                            src_t, scal = S[i], acc[:, i : i + 1]
                        sn = gpool.tile([P, O], sdt, name="st")
                        ceng = nc.vector
                        if variant == "mx1" and i % 4 != 0:
                            ceng = nc.gpsimd
                        ceng.scalar_tensor_tensor(
                            out=sn[:, :],
                            in0=src_t[:, :],
                            scalar=scal,
                            in1=src_t[:, :],
                            op0=Alu.subtract,
                            op1=Alu.mult,
                            accum_out=accn[:, i : i + 1],
                        )
                    else:
                        fc = gpool.tile([P, O], sdt, name="st")
                        nc.vector.tensor_scalar(
                            fc[:, :], S[i][:, :], gi, None, Alu.mult
                        )
                        sn = gpool.tile([P, O], sdt, name="st")
                        nc.vector.scalar_tensor_tensor(
                            out=sn[:, :],
                            in0=fc[:, :],
                            scalar=1.0,
                            in1=fc[:, :],
                            op0=Alu.subtract,
                            op1=Alu.mult,
                            accum_out=accn[:, i : i + 1],
                        )
                    S[i] = sn
                acc = accn
            if variant == "probe":
                # probe: stride-2 fp16 write cost (for pair-pack evaluation)
                pb = gpool.tile([P, O, 2], f16, name="pb", bufs=2)
                nc.vector.tensor_scalar(
                    pb[:, :, 0], S[0][:, :], gam[:, 0:1], None, Alu.mult
                )
                nc.vector.tensor_scalar(
                    pb[:, :, 1], S[1][:, :], gam[:, 1:2], None, Alu.mult
                )
                nc.scalar.mul(pb[:, :, 0], S[2][:, :], gam[:, 2:3])
            for i in range(WV):
                t = g * WV + i
                if variant == "hybr5":
                    nc.sync.dma_start(
                        out=w[t * P : (t + 1) * P, 0 : O * K // 2],
                        in_=outs[i][:, : O // 2, :],
                    )
                    nc.gpsimd.dma_start(
                        out=w[t * P : (t + 1) * P, O * K // 2 :],
                        in_=outs[i][:, O // 2 :, :],
                    )
                else:
                    nc.sync.dma_start(
                        out=w[t * P : (t + 1) * P, :], in_=outs[i][:, :, :]
                    )

        emit_exps(0)
        for g in range(WAVES):
            emit_rounds(g)

    nc.compile()
    return nc


def _get_nc(variant=None):
    variant = variant or VARIANT
    if variant not in _cached:
        _cached[variant] = _build(variant)
    return _cached[variant]


def _make_in_maps(D, log_temp):
    Dr = np.ascontiguousarray(np.asarray(D, dtype=np.float32).reshape(ROWS, O))
    lt = np.asarray(log_temp, dtype=np.float32).reshape(1, 1)
    return [
        {"d": Dr[c * RPC : (c + 1) * RPC], "log_temp": lt}
        for c in range(N_CORES)
    ]


def _gather(results):
    parts = [
        np.asarray(results[c]["w"], dtype=np.float32).reshape(RPC, O, K)
        for c in range(N_CORES)
    ]
    return np.concatenate(parts, axis=0).reshape(B, M, O, K)


def run_spmd(D, log_temp, trace=False, variant=None, **kwargs):
    """Run on all 8 cores; returns (W, BassKernelResults)."""
    from concourse.bass_utils import run_bass_kernel_spmd

    nc = _get_nc(variant)
    res = run_bass_kernel_spmd(
        nc, _make_in_maps(D, log_temp), list(range(N_CORES)), trace=trace, **kwargs
    )
    return _gather(res.results), res


def kernel(D, log_temp):
    W, _ = run_spmd(D, log_temp)
    return W


# revision 27
# speedup vs baseline: 1.2934x; 1.0053x over previous
"""Trainium2 Bass kernel: NeuralNearestNeighbors continuous-KNN weight volumes.

Reference computation (per row of D.reshape(b*m, o), K=8 rounds):
    logits = D / exp(log_temp)
    for k in range(K):
        w_k = log_softmax(logits);  out_k = exp(w_k)
        logits = logits + log1mexp(w_k)          # log(1 - p_k)
    W = stack(out_k, axis=-1)                     # (b, m, o, K)

Scale-invariant recurrence used on device: keep a state S that is an
arbitrary per-row scalar multiple of the round's softmax weights F_k,
with its true row-sum `a = sum(S)` tracked by each op's accumulator.
Then F_k = S*(1/a) always (since sum(F_k) == 1), and

    S' = (S - a)*S          a' = sum(S')

reproduces the reference exactly (S' = a^2*(F-F^2), and the scale a^2
cancels in the next normalization).  One native DVE scalar_tensor_tensor
per round with the accumulator fed back as the per-partition scalar -
no reciprocal on the chain.  The state squares each round, so it is
rescaled to F (one tensor_scalar by gam=1/a) at rounds {1,4} to stay in
fp16 range; after a rescale |S| <= ~50 for the next two rounds.

Engine/dtype assignment (measured on HW, per [128, 512] op):
  - chain stt fp16 contiguous on DVE: ~729 ns (2x 2-byte DVE mode)
  - strided (stride-8) f32 output writes: DVE tensor_scalar ~683 ns,
    ACT activation-copy ~1270 ns; split ACT:DVE = 23:9 per wave of 4
    tiles x 8 rounds.  fp16 strided writes are 2-4x slower than f32 on
    both engines, and GpSimd strided writes are ~7.8 us - never used.
  - exp on ACT (fp16 out, f32 accum, bias -6 bounds pre-rescale growth)
  - batched [P,4] reciprocals per wave round on DVE (off-chain, feeds
    only the strided output writes).
Waves of 4 tiles are software-pipelined (next wave's exps are emitted
mid-wave) so ACT/DVE stay >90% busy.  Output tile stays f32 (fp16
strided writes would cost more engine time than the DMA saves).

Sharding: purely rowwise data-parallel over b*m = 16384 rows; 2048 rows
per core across 8 cores; log_temp replicated.  HW exec ~152 us vs 303 us
baseline; DVE ~138 us / ACT ~145 us busy, DMA ~115 us.
"""

import os

import numpy as np

B, M, O = 16, 1024, 512
K = 8
N_CORES = 8
ROWS = B * M                     # 16384
RPC = ROWS // N_CORES            # 2048 rows per core
P = 128
TILES = RPC // P                 # 16 row-tiles per core
WV = 4                           # tiles per wave
WAVES = TILES // WV

VARIANT = os.environ.get("KVAR", "hybr2")

# variant -> (state_dtype, out_dtype, chain, pass1 engine counts (A, P, D))
_CFG = {
    "f32a": ("f32", "f32", "amr", (20, 6, 6)),
    "f32n": ("f32", "f32", "natr", (20, 6, 6)),
    "mx1": ("f32", "f32", "natr", (16, 0, 16)),
    "h16a": ("f16", "f16", "amr", (15, 13, 4)),
    "h16n": ("f16", "f16", "nat", (26, 0, 6)),
    "hyb": ("f16", "f32", "nat", (15, 0, 17)),
    "hybr": ("f16", "f32", "natr", (11, 0, 21)),
    "hybr2": ("f16", "f32", "natr", (23, 0, 9)),
    "hybr3": ("f16", "f32", "natr", (22, 0, 10)),
    "final": ("f16", "f32", "natr", (21, 0, 11)),
    "hybr4": ("f16", "f32", "natr", (24, 0, 8)),
    "hybr5": ("f16", "f32", "natr", (23, 0, 9)),
    "b16": ("b16", "f32", "natr", (23, 0, 9)),
    "sq": ("f16", "f32", "natr", (17, 0, 15)),
    "sq1": ("f16", "f32", "natr", (20, 0, 12)),
    "pe": ("f16", "f16", "pe", (27, 0, 5)),
}

_cached = {}


def _make_pattern(n_act, n_pool, n_dve, total):
    """Largest-remainder round-robin spread of engine codes over slots."""
    pools = [("A", n_act), ("P", n_pool), ("D", n_dve)]
    credit = {c: 0.0 for c, _ in pools}
    out = []
    for _ in range(total):
        for c, n in pools:
            credit[c] += n / total
        pick = max(credit, key=lambda c: credit[c])
        credit[pick] -= 1.0
        out.append(pick)
    return out


def _build(variant):
    from contextlib import ExitStack

    import concourse.bacc as bacc
    import concourse.tile as tile
    from concourse import mybir

    f32 = mybir.dt.float32
    f16 = mybir.dt.float16
    Alu = mybir.AluOpType
    Act = mybir.ActivationFunctionType

    sdt_s, odt_s, chain, counts = _CFG[variant]
    sdt = f16 if sdt_s == "f16" else (mybir.dt.bfloat16 if sdt_s == "b16" else f32)
    odt = f16 if odt_s == "f16" else f32
    pat = _make_pattern(*counts, total=K * WV)

    nc = bacc.Bacc(
        "TRN2",
        target_bir_lowering=False,
        debug=False,
        enable_asserts=False,
        num_devices=N_CORES,
    )
    d = nc.dram_tensor("d", [RPC, O], f32, kind="ExternalInput").ap()
    lt = nc.dram_tensor("log_temp", [1, 1], f32, kind="ExternalInput").ap()
    w = nc.dram_tensor("w", [RPC, O * K], odt, kind="ExternalOutput").ap()

    with tile.TileContext(nc) as tc, ExitStack() as ctx:
        singles = ctx.enter_context(tc.tile_pool(name="singles", bufs=1))
        dpool = ctx.enter_context(tc.tile_pool(name="dslab", bufs=1))
        gpool = ctx.enter_context(
            tc.tile_pool(name="state", bufs=20 if variant in ("sq", "sq1") else (28 if chain in ("nat", "natr") else 16))
        )
        outp = ctx.enter_context(
            tc.tile_pool(name="out", bufs=10 if odt == f16 else 7)
        )
        small = ctx.enter_context(tc.tile_pool(name="small", bufs=64))
        if chain == "pe":
            fpool = ctx.enter_context(tc.tile_pool(name="fbuf", bufs=72))
            pspool = ctx.enter_context(tc.psum_pool(name="ps", bufs=2))

        # log_temp -> 1/T = exp(-log_temp), replicated to all 128 partitions.
        lt_sb = singles.tile([P, 1], f32)
        nc.sync.dma_start(out=lt_sb[:, :], in_=lt.to_broadcast((P, 1)))
        invt = singles.tile([P, 1], f32)
        nc.scalar.activation(invt[:, :], lt_sb[:, :], Act.Exp, scale=-1.0)
        bias6 = singles.tile([P, 1], f32)
        nc.vector.memset(bias6[:, :], -6.0)
        biash = singles.tile([P, 1], f32)
        nc.vector.memset(biash[:, :], -0.5)
        if chain == "pe":
            # fp16 identity for PE copy-through: I[p, j] = (p == j)
            ident = singles.tile([P, P], f16)
            nc.gpsimd.memset(ident[:, :], 1.0)
            nc.gpsimd.affine_select(
                out=ident[:, :],
                in_=ident[:, :],
                compare_op=Alu.is_equal,
                fill=0.0,
                base=0,
                pattern=[[-1, P]],
                channel_multiplier=1,
            )

        din = d.rearrange("(t p) o -> p t o", p=P)
        dslab = dpool.tile([P, TILES, O], f32)
        if variant == "hybr5":
            nc.gpsimd.dma_start(out=dslab[:, 0:1, :], in_=din[:, 0:1, :])
            nc.gpsimd.dma_start(out=dslab[:, 1:WV, :], in_=din[:, 1:WV, :])
        for g in range(1 if variant == "hybr5" else 0, WAVES):
            # SWDGE path keeps the HWDGE rings free for output writes.
            nc.gpsimd.dma_start(
                out=dslab[:, g * WV : (g + 1) * WV, :],
                in_=din[:, g * WV : (g + 1) * WV, :],
            )

        wave_state = {}

        def emit_exps(g):
            """exp round for wave g: S_0 = exp(D/T), acc = row sums."""
            acc = small.tile([P, WV], f32)
            S = []
            for i in range(WV):
                t = g * WV + i
                s0 = gpool.tile([P, O], sdt, name="st")
                nc.scalar.activation(
                    s0[:, :],
                    dslab[:, t, :],
                    Act.Exp,
                    scale=invt[:, :],
                    bias=bias6[:, :],
                    accum_out=acc[:, i : i + 1],
                )
                S.append(s0)
            wave_state[g] = (S, acc)

        def emit_rounds_pe(g):
            S, acc = wave_state.pop(g)
            fcs = [[None] * K for _ in range(WV)]
            for r in range(K):
                gam = small.tile([P, WV], f32)
                nc.vector.reciprocal(gam[:, :], acc[:, :])
                for i in range(WV):
                    gi = gam[:, i : i + 1]
                    fc = fpool.tile([P, O], f16, name="fc")
                    e = pat[r * WV + i]
                    if e == "D":
                        nc.vector.tensor_scalar(
                            fc[:, :], S[i][:, :], gi, None, Alu.mult
                        )
                    else:
                        nc.scalar.mul(fc[:, :], S[i][:, :], gi)
                    fcs[i][r] = fc
                if r == 2 and g + 1 < WAVES:
                    emit_exps(g + 1)
                if r == K - 1:
                    break
                accn = small.tile([P, WV], f32)
                for i in range(WV):
                    # chain: S' = (S - a)*S with accumulator feedback; at
                    # rounds 1 and 4 restart from the normalized F (already
                    # materialized for the PE) to keep fp16 bounded.
                    if r in (1, 4):
                        src_t, scal = fcs[i][r], 1.0
                    else:
                        src_t, scal = S[i], acc[:, i : i + 1]
                    sn = gpool.tile([P, O], sdt, name="st")
                    nc.vector.scalar_tensor_tensor(
                        out=sn[:, :],
                        in0=src_t[:, :],
                        scalar=scal,
                        in1=src_t[:, :],
                        op0=Alu.subtract,
                        op1=Alu.mult,
                        accum_out=accn[:, i : i + 1],
                    )
                    S[i] = sn
                acc = accn
            for i in range(WV):
                t = g * WV + i
                ot = outp.tile([P, O, K], odt, name="ot")
                for h in range(2):
                    ph = pspool.tile([P, O // 2, K], f32, name="ph")
                    for k in range(K):
                        nc.tensor.matmul(
                            out=ph[:, :, k],
                            lhsT=ident[:, :],
                            rhs=fcs[i][k][:, h * (O // 2) : (h + 1) * (O // 2)],
                            start=True,
                            stop=True,
                        )
                    # PSUM -> SBUF fp16 downcast on the idle GpSimd engine
                    nc.gpsimd.tensor_copy(
                        ot[:, h * (O // 2) : (h + 1) * (O // 2), :],
                        ph[:, :, :],
                    )
                nc.sync.dma_start(
                    out=w[t * P : (t + 1) * P, :], in_=ot[:, :, :]
                )

        def emit_rounds_sq(g):
            """ACT-square chain wave: state v = (F-0.5)^2 (f32), G' = 0.25-v.
            Round update is one ACT Square op with [P,1] scale/bias APs; the
            whole recurrence runs on ACT, freeing DVE for strided writes."""
            S, acc = wave_state.pop(g)
            patq = _make_pattern(11, 0, 21, K * WV)
            outs = [outp.tile([P, O, K], odt, name="ot") for _ in range(WV)]
            V = [None] * WV
            for r in range(K):
                if r == 0:
                    gam = small.tile([P, WV], f32)
                    nc.vector.reciprocal(gam[:, :], acc[:, :])
                else:
                    # acc holds sum(v); a' = 128 - sum(v); use gneg = -1/a'
                    am = small.tile([P, WV], f32)
                    nc.vector.tensor_scalar(
                        am[:, :], acc[:, :], 1.0, -128.0, Alu.mult, Alu.add
                    )
                    gam = small.tile([P, WV], f32)
                    nc.vector.reciprocal(gam[:, :], am[:, :])  # = -1/a'
                    # pass1 offset t1 = 0.25*gam' = -0.25*gneg; square bias
                    # bg = t1 - 0.5
                    t1 = small.tile([P, WV], f32)
                    nc.vector.tensor_scalar(
                        t1[:, :], gam[:, :], -0.25, None, Alu.mult
                    )
                    bg = small.tile([P, WV], f32)
                    nc.vector.tensor_scalar(
                        bg[:, :], t1[:, :], 1.0, -0.5, Alu.mult, Alu.add
                    )
                for i in range(WV):
                    f = outs[i][:, :, r]
                    gi = gam[:, i : i + 1]
                    e = patq[r * WV + i]
                    if r == 0:
                        # F_0 = S*gam
                        if e == "D":
                            nc.vector.tensor_scalar(
                                f, S[i][:, :], gi, None, Alu.mult
                            )
                        else:
                            nc.scalar.mul(f, S[i][:, :], gi)
                    else:
                        # F_r = (0.25 - v)*gam' = v*gneg + t1; Copy cannot
                        # take an AP bias, so these always run on DVE
                        nc.vector.tensor_scalar(
                            f, V[i][:, :], gi, t1[:, i : i + 1],
                            Alu.mult, Alu.add,
                        )
                if r == 2 and g + 1 < WAVES:
                    emit_exps(g + 1)
                if r == K - 1:
                    break
                accn = small.tile([P, WV], f32)
                for i in range(WV):
                    # v' = (F - 0.5)^2, accum sum(v'); F expressed via the
                    # same scale/bias as pass1 (r=0: S*gam - 0.5)
                    vn = gpool.tile([P, O], f32, name="vt", bufs=10)
                    if r == 0:
                        nc.scalar.activation(
                            vn[:, :], S[i][:, :], Act.Square,
                            bias=biash[:, :], scale=gam[:, i : i + 1],
                            accum_out=accn[:, i : i + 1],
                        )
                    else:
                        nc.scalar.activation(
                            vn[:, :], V[i][:, :], Act.Square,
                            bias=bg[:, i : i + 1], scale=gam[:, i : i + 1],
                            accum_out=accn[:, i : i + 1],
                        )
                    V[i] = vn
                acc = accn
            for i in range(WV):
                t = g * WV + i
                nc.sync.dma_start(
                    out=w[t * P : (t + 1) * P, :], in_=outs[i][:, :, :]
                )

        def emit_rounds(g):
            if chain == "pe":
                return emit_rounds_pe(g)
            if (variant == "sq" and g % 2 == 1) or (variant == "sq1" and g == 1):
                return emit_rounds_sq(g)
            S, acc = wave_state.pop(g)
            outs = [outp.tile([P, O, K], odt, name="ot") for _ in range(WV)]
            for r in range(K):
                gam = small.tile([P, WV], f32)
                nc.vector.reciprocal(gam[:, :], acc[:, :])
                for i in range(WV):
                    f = outs[i][:, :, r]
                    gi = gam[:, i : i + 1]
                    e = pat[r * WV + i]
                    if e == "A":
                        nc.scalar.mul(f, S[i][:, :], gi)
                    elif e == "D":
                        nc.vector.tensor_scalar(f, S[i][:, :], gi, None, Alu.mult)
                    else:
                        nc.gpsimd.tensor_scalar(f, S[i][:, :], gi, None, Alu.mult)
                if r == 2 and g + 1 < WAVES:
                    # software pipeline: next wave's exps land on ACT now so
                    # its first reciprocal is ready at this wave's end.
                    emit_exps(g + 1)
                if r == K - 1:
                    break
                accn = small.tile([P, WV], f32)
                for i in range(WV):
                    gi = gam[:, i : i + 1]
                    if chain == "amr":
                        sn = gpool.tile([P, O], sdt, name="st")
                        nc.vector.affine_mul_reduce(
                            out=sn[:, :],
                            accum_out=accn[:, i : i + 1],
                            in0=S[i][:, :],
                            in1=S[i][:, :],
                            scale=gi,
                            bias=-1.0,
                        )
                    elif chain == "natr":
                        # native chain: S' = (S - a)*S is exact for any state
                        # scale; rescale keeps the state's squared-growth
                        # bounded (f32: once; fp16: twice, max |S| ~50).
                        rescale_rounds = (1, 4) if sdt == f16 else (2,)
                        if r in rescale_rounds:
                            fc = gpool.tile([P, O], sdt, name="st")
                            reng = nc.vector
                            reng.tensor_scalar(
                                fc[:, :], S[i][:, :], gi, None, Alu.mult
                            )
                            src_t, scal = fc, 1.0
                        else:
                            src_t, scal = S[i], acc[:, i : i + 1]
                        sn = gpool.tile([P, O], sdt, name="st")
                        ceng = nc.vector
                        if variant == "mx1" and i % 4 != 0:
                            ceng = nc.gpsimd
                        ceng.scalar_tensor_tensor(
                            out=sn[:, :],
                            in0=src_t[:, :],
                            scalar=scal,
                            in1=src_t[:, :],
                            op0=Alu.subtract,
                            op1=Alu.mult,
                            accum_out=accn[:, i : i + 1],
                        )
                    else:
                        fc = gpool.tile([P, O], sdt, name="st")
                        nc.vector.tensor_scalar(
                            fc[:, :], S[i][:, :], gi, None, Alu.mult
                        )
                        sn = gpool.tile([P, O], sdt, name="st")
                        nc.vector.scalar_tensor_tensor(
                            out=sn[:, :],
                            in0=fc[:, :],
                            scalar=1.0,
                            in1=fc[:, :],
                            op0=Alu.subtract,
                            op1=Alu.mult,
                            accum_out=accn[:, i : i + 1],
                        )
                    S[i] = sn
                acc = accn
            if variant == "probe":
                # probe: stride-2 fp16 write cost (for pair-pack evaluation)
                pb = gpool.tile([P, O, 2], f16, name="pb", bufs=2)
                nc.vector.tensor_scalar(
                    pb[:, :, 0], S[0][:, :], gam[:, 0:1], None, Alu.mult
                )
                nc.vector.tensor_scalar(
                    pb[:, :, 1], S[1][:, :], gam[:, 1:2], None, Alu.mult
                )
                nc.scalar.mul(pb[:, :, 0], S[2][:, :], gam[:, 2:3])
            for i in range(WV):
                t = g * WV + i
                if variant == "hybr5":
                    nc.sync.dma_start(
                        out=w[t * P : (t + 1) * P, 0 : O * K // 2],
                        in_=outs[i][:, : O // 2, :],
                    )
                    nc.gpsimd.dma_start(
                        out=w[t * P : (t + 1) * P, O * K // 2 :],
                        in_=outs[i][:, O // 2 :, :],
                    )
                else:
                    nc.sync.dma_start(
                        out=w[t * P : (t + 1) * P, :], in_=outs[i][:, :, :]
                    )

        emit_exps(0)
        for g in range(WAVES):
            emit_rounds(g)

    nc.compile()
    return nc


def _get_nc(variant=None):
    variant = variant or VARIANT
    if variant not in _cached:
        _cached[variant] = _build(variant)
    return _cached[variant]


def _make_in_maps(D, log_temp):
    Dr = np.ascontiguousarray(np.asarray(D, dtype=np.float32).reshape(ROWS, O))
    lt = np.asarray(log_temp, dtype=np.float32).reshape(1, 1)
    return [
        {"d": Dr[c * RPC : (c + 1) * RPC], "log_temp": lt}
        for c in range(N_CORES)
    ]


def _gather(results):
    parts = [
        np.asarray(results[c]["w"], dtype=np.float32).reshape(RPC, O, K)
        for c in range(N_CORES)
    ]
    return np.concatenate(parts, axis=0).reshape(B, M, O, K)


def run_spmd(D, log_temp, trace=False, variant=None, **kwargs):
    """Run on all 8 cores; returns (W, BassKernelResults)."""
    from concourse.bass_utils import run_bass_kernel_spmd

    nc = _get_nc(variant)
    res = run_bass_kernel_spmd(
        nc, _make_in_maps(D, log_temp), list(range(N_CORES)), trace=trace, **kwargs
    )
    return _gather(res.results), res


def kernel(D, log_temp):
    W, _ = run_spmd(D, log_temp)
    return W
